# revision 14
# baseline (speedup 1.0000x reference)
"""AdaLN (DiT-style) transformer block on 8 Trainium2 NeuronCores.

Data-parallel over batch: core b computes batch element b end-to-end
(B == n_cores == 8), no collectives. bf16 datapath (weights + activations),
fp32 PSUM accumulation and residuals.

Key structure per core:
  - AdaLN modulate folded into weights: W'_qkv = qkv_wT * (1+s1) rows,
    bias row = b1 @ qkv_wT; same for mlp1 with s2/b2; gate g1/g2 folded
    into proj/mlp2 weight columns. LN feeds transposes independent of cond.
  - Attention transposed S^T[j,i], head pairs packed via tile_position
    (rows 0:64 / 64:128 concurrent) into one shared PSUM tile; softmax exp
    split between scalar engine (exact Exp) and vector engine (Schraudolph
    int16->bf16 bitcast); PV ones-column denominators; 4-bank attention
    PSUM leaves room to absorb qkT emission as real PE work between
    attention steps (keeps the HAM clock gate open).
  - proj + residual + LN2 per-tile pipeline; MLP hidden transposed; silu
    on scalar overlaps PE.

Fixed problem shape: x [8, 1024, 384], cond [8, 384], H=6 heads, hd=64.
"""
import sys

if '/opt/trn_rl_repo' not in sys.path:
    sys.path.insert(0, '/opt/trn_rl_repo')

import numpy as np
import ml_dtypes

import concourse.bacc as bacc
import concourse.tile as tile
from concourse import masks, mybir
from concourse.bass_utils import run_bass_kernel_spmd

B, L, D, H = 8, 1024, 384, 6
HD = D // H                  # 64
DQ = 3 * D                   # 1152
DM = 4 * D                   # 1536
DC = 6 * D                   # 2304
KD = D // 128                # 3 k-tiles over D
IT = L // 128                # 8 i-tiles over L
IC = L // 512                # 2 512-chunks over L
SCALE = HD ** -0.5
EPS = 1e-5
# Schraudolph exp -> bf16 bits: bits16 = round(A*score + B16)
A_SCH = SCALE * 128.0 / np.log(2.0)
B_SCH = 127.0 * 128.0 - 5.585

f32 = mybir.dt.float32
bf16 = mybir.dt.bfloat16
i16 = mybir.dt.int16
ACTF = mybir.ActivationFunctionType
ALU = mybir.AluOpType

_cache = {}


def _layernorm(nc, sb, xt, eps_t, out):
    """out(bf16) = LN(xt) over the free dim, no affine."""
    stats = sb.tile([128, 6], f32, name="ln_stats", tag="ln_stats")
    nc.vector.bn_stats(out=stats, in_=xt)
    mv = sb.tile([128, 2], f32, name="ln_mv", tag="ln_mv")
    nc.vector.bn_aggr(out=mv, in_=stats)
    rstd = sb.tile([128, 1], f32, name="ln_rstd", tag="ln_rstd")
    nc.scalar.activation(out=rstd, in_=mv[:, 1:2], func=ACTF.Sqrt, bias=eps_t, scale=1.0)
    nc.vector.reciprocal_approx_fast(rstd, rstd)
    negmr = sb.tile([128, 1], f32, name="ln_negmr", tag="ln_negmr")
    nc.gpsimd.tensor_scalar(out=negmr, in0=mv[:, 0:1], scalar1=rstd, scalar2=-1.0,
                            op0=ALU.mult, op1=ALU.mult)
    nc.gpsimd.tensor_scalar(out=out, in0=xt, scalar1=rstd, scalar2=negmr,
                            op0=ALU.mult, op1=ALU.add)


def build(flags):
    """Per-core Bass program. flags: (cond_b, qkv_b, proj_b, m1b, m2b) nonzero."""
    use_cb, use_qb, use_pb, use_m1, use_m2 = flags
    nc = bacc.Bacc()

    xb = nc.declare_dram_parameter("xb", [L, D], f32, isOutput=False)
    cond = nc.declare_dram_parameter("cond", [D], f32, isOutput=False)
    cond_wT = nc.declare_dram_parameter("cond_wT", [D, DC], bf16, isOutput=False)
    qkv_wT = nc.declare_dram_parameter("qkv_wT", [D, DQ], bf16, isOutput=False)
    proj_wT = nc.declare_dram_parameter("proj_wT", [D, D], bf16, isOutput=False)
    w1T = nc.declare_dram_parameter("w1T", [D, DM], bf16, isOutput=False)
    w2T = nc.declare_dram_parameter("w2T", [DM, D], bf16, isOutput=False)
    if use_cb:
        cond_b = nc.declare_dram_parameter("cond_b", [DC], f32, isOutput=False)
    if use_qb:
        qkv_b = nc.declare_dram_parameter("qkv_b", [DQ], f32, isOutput=False)
    if use_pb:
        proj_b = nc.declare_dram_parameter("proj_b", [D], f32, isOutput=False)
    if use_m1:
        mlp_b1 = nc.declare_dram_parameter("mlp_b1", [DM], f32, isOutput=False)
    if use_m2:
        mlp_b2 = nc.declare_dram_parameter("mlp_b2", [D], f32, isOutput=False)
    out = nc.declare_dram_parameter("out", [L, D], f32, isOutput=True)

    with tile.TileContext(nc) as tc:
        from contextlib import ExitStack
        ctx = ExitStack()
        with ctx:
            persist = ctx.enter_context(tc.tile_pool(name="persist", bufs=1))
            sb = ctx.enter_context(tc.tile_pool(name="small", bufs=4))
            hpool = ctx.enter_context(tc.tile_pool(name="hpool", bufs=4))
            dramp = ctx.enter_context(tc.tile_pool(name="dramp", bufs=1, space="DRAM"))

            # ---------------- DMAs (split across SP + ACT queues) ----------
            cvec = persist.tile([128, KD], f32, tag="cvec")
            nc.sync.dma_start(out=cvec, in_=cond[:].rearrange("(k p) -> p k", p=128))
            xall = persist.tile([128, IT * D], f32, tag="xall")
            nc.sync.dma_start(out=xall[:, 0:4 * D].rearrange("p (i f) -> p i f", f=D),
                              in_=xb[0:512, :].rearrange("(i p) f -> p i f", p=128))
            nc.sync.dma_start(out=xall[:, 4 * D:].rearrange("p (i f) -> p i f", f=D),
                              in_=xb[512:1024, :].rearrange("(i p) f -> p i f", p=128))
            xt = [xall[:, i * D:(i + 1) * D] for i in range(IT)]

            early_cm = tc.tile_pool(name="early", bufs=1)
            early = early_cm.__enter__()
            condw_sb = early.tile([128, KD * DC], bf16, tag="condw")
            nc.scalar.dma_start(out=condw_sb[:, :].rearrange("p (k f) -> p k f", f=DC),
                                in_=cond_wT[:, :].rearrange("(k p) f -> p k f", p=128))
            qkvw_sb = early.tile([128, KD * DQ], bf16, tag="qkvw")
            nc.scalar.dma_start(out=qkvw_sb[:, :].rearrange("p (k f) -> p k f", f=DQ),
                                in_=qkv_wT[:, :].rearrange("(k p) f -> p k f", p=128))
            w1_raw = early.tile([128, KD * DM], bf16, tag="w1raw")
            nc.sync.dma_start(out=w1_raw[:, :].rearrange("p (k f) -> p k f", f=DM),
                              in_=w1T[:, :].rearrange("(k p) f -> p k f", p=128))
            projw_raw = early.tile([128, KD * D], bf16, tag="projw_raw")
            nc.sync.dma_start(out=projw_raw[:, :].rearrange("p (k f) -> p k f", f=D),
                              in_=proj_wT[:, :].rearrange("(k p) f -> p k f", p=128))
            w2_raw = persist.tile([128, 12 * D], bf16, tag="w2raw")
            nc.sync.dma_start(out=w2_raw[:, :].rearrange("p (k f) -> p k f", f=D),
                              in_=w2T[:, :].rearrange("(k p) f -> p k f", p=128))

            # ---------------- constants ----------------
            identb = persist.tile([128, 128], bf16, tag="identb")
            masks.make_identity(nc, identb[:, :])
            identf = persist.tile([128, 128], f32, tag="identf")
            nc.gpsimd.tensor_copy(identf, identb)
            eps_t = persist.tile([128, 1], f32, tag="eps")
            nc.vector.memset(eps_t, EPS)
            ones6_f = persist.tile([128, 6, 1], bf16, tag="ones6f")
            nc.vector.memset(ones6_f, 1.0)

            ps_pre_cm = tc.tile_pool(name="ps_pre", bufs=2, space="PSUM")
            ps_pre = ps_pre_cm.__enter__()
            # PE warmup while first DMAs land
            warm_ps = ps_pre.tile([128, 128], f32, name="warm_ps", tag="pt")
            for w in range(18):
                nc.tensor.matmul(warm_ps[:, :], identb[:, :], identb[:, :],
                                 start=(w == 0), stop=(w == 17))
            wsink = sb.tile([1, 1], f32, name="wsink", tag="wsink")
            nc.scalar.copy(wsink, warm_ps[0:1, 0:1])
            wdram = dramp.tile([1, 1], f32, name="wdram", tag="wdram")
            nc.sync.dma_start(out=wdram, in_=wsink)
            # preload Sqrt/Exp activation tables off the critical path
            tdum = sb.tile([1, 1], f32, name="tdum", tag="tdum")
            nc.scalar.activation(out=tdum, in_=eps_t[0:1, :], func=ACTF.Sqrt)
            nc.scalar.activation(out=tdum, in_=eps_t[0:1, :], func=ACTF.Exp)

            # ---------------- LN1 + transposes (x-dependent only) ----------
            lnT = [persist.tile([128, L], bf16, name=f"lnT{k}", tag=f"lnT{k}")
                   for k in range(KD)]
            copy_engs = [nc.vector.tensor_copy, nc.scalar.copy]
            for i in range(IT):
                ln = hpool.tile([128, D], bf16, name="ln1", tag="h1")
                _layernorm(nc, sb, xt[i], eps_t, ln)
                for k in range(KD):
                    pt = ps_pre.tile([128, 128], bf16, name="pt", tag="pt")
                    nc.tensor.transpose(pt[:, :], ln[:, k * 128:(k + 1) * 128],
                                        identb[:, :])
                    copy_engs[(i + k) % 2](lnT[k][:, i * 128:(i + 1) * 128], pt[:, :])

            # ---------------- conditioning ----------------
            scond = persist.tile([128, KD], bf16, tag="scond")
            nc.scalar.activation(out=scond, in_=cvec, func=ACTF.Silu)
            s1p1 = persist.tile([128, KD], f32, tag="s1p1")
            s2p1 = persist.tile([128, KD], f32, tag="s2p1")
            b1c = persist.tile([128, KD], bf16, tag="b1c")
            b2c = persist.tile([128, KD], bf16, tag="b2c")
            g1bc = persist.tile([128, D], f32, tag="g1bc")
            g2bc = persist.tile([128, D], f32, tag="g2bc")
            col_dst = {0: s1p1, 1: b1c, 3: s2p1, 4: b2c}
            for c in (0, 1, 2, 3, 4, 5):
                pc = ps_pre.tile([1, D], f32, name="pc", tag="pq")
                for k in range(KD):
                    nc.tensor.matmul(pc[:, :], scond[:, k:k + 1],
                                     condw_sb[:, (k * DC + c * D):(k * DC + (c + 1) * D)],
                                     start=(k == 0), stop=(k == KD - 1))
                cseg = sb.tile([1, D], f32, name="cseg", tag="cseg")
                nc.scalar.copy(cseg, pc[:, :])
                if use_cb:
                    cbseg = sb.tile([1, D], f32, name="cbseg", tag="cbseg")
                    nc.sync.dma_start(out=cbseg,
                                      in_=cond_b[c * D:(c + 1) * D].rearrange("(o f) -> o f", o=1))
                    nc.vector.tensor_add(cseg, cseg, cbseg)
                if c in (0, 3):
                    nc.scalar.add(cseg, cseg, 1.0)   # 1 + scale
                if c in col_dst:
                    csegb = sb.tile([1, D], bf16, name="csegb", tag="csegb")
                    nc.vector.tensor_copy(csegb, cseg)
                    for k in range(KD):
                        ptc = ps_pre.tile([128, 1], bf16, name="ptc", tag="pt")
                        nc.tensor.transpose(ptc[:, :], csegb[0:1, k * 128:(k + 1) * 128],
                                            identb[0:1, 0:1])
                        nc.vector.tensor_copy(col_dst[c][:, k:k + 1], ptc[:, :])
                else:
                    nc.gpsimd.partition_broadcast(g1bc if c == 2 else g2bc, cseg[:1, :])

            # scaled weights W' = W * (1+s) rows; gated weights W * g cols
            qkvw_sc = persist.tile([128, KD * DQ], bf16, tag="qkvw_sc")
            for k in range(KD):
                nc.vector.tensor_scalar(out=qkvw_sc[:, k * DQ:(k + 1) * DQ],
                                        in0=qkvw_sb[:, k * DQ:(k + 1) * DQ],
                                        scalar1=s1p1[:, k:k + 1], scalar2=None,
                                        op0=ALU.mult)
            projw_sc = persist.tile([128, KD * D], bf16, tag="projw_sc")
            for k in range(KD):
                nc.gpsimd.tensor_mul(projw_sc[:, k * D:(k + 1) * D],
                                     projw_raw[:, k * D:(k + 1) * D], g1bc)
            # rowq = b1 @ qkv_wT (+ qkv_b): [1, DQ]
            rowq = persist.tile([1, DQ], f32, tag="rowq")
            for cc in range(3):
                pr = ps_pre.tile([1, D], f32, name="pr", tag="pq")
                for k in range(KD):
                    nc.tensor.matmul(pr[:, :], b1c[:, k:k + 1],
                                     qkvw_sb[:, (k * DQ + cc * D):(k * DQ + (cc + 1) * D)],
                                     start=(k == 0), stop=(k == KD - 1))
                nc.scalar.copy(rowq[:, cc * D:(cc + 1) * D], pr[:, :])
            if use_qb:
                qbrow = sb.tile([1, DQ], f32, name="qbrow", tag="qbrow")
                nc.sync.dma_start(out=qbrow, in_=qkv_b[:].rearrange("(o f) -> o f", o=1))
                nc.vector.tensor_add(rowq, rowq, qbrow)
            qkcols = persist.tile([128, 6], f32, tag="qkcols")
            for t in range(6):
                ptc = ps_pre.tile([128, 1], f32, name="ptq", tag="pt")
                nc.tensor.transpose(ptc[:, :], rowq[0:1, t * 128:(t + 1) * 128],
                                    identf[0:1, 0:1])
                nc.vector.tensor_copy(qkcols[:, t:t + 1], ptc[:, :])
            vb_bc = persist.tile([128, D], f32, tag="vb_bc")
            nc.gpsimd.partition_broadcast(vb_bc, rowq[:1, 768:1152])

            # mlp weight prep (consumed later; vector/gpsimd slack)
            w1_sc = persist.tile([128, KD * DM], bf16, tag="w1_sc")
            for k in range(KD):
                nc.vector.tensor_scalar(out=w1_sc[:, k * DM:(k + 1) * DM],
                                        in0=w1_raw[:, k * DM:(k + 1) * DM],
                                        scalar1=s2p1[:, k:k + 1], scalar2=None,
                                        op0=ALU.mult)
            w2_sc = persist.tile([128, 12 * D], bf16, tag="w2_sc")
            for k in range(12):
                nc.gpsimd.tensor_mul(w2_sc[:, k * D:(k + 1) * D],
                                     w2_raw[:, k * D:(k + 1) * D], g2bc)
            rowm = persist.tile([1, DM], f32, tag="rowm")
            for cc in range(4):
                pr = ps_pre.tile([1, D], f32, name="prm", tag="pq")
                for k in range(KD):
                    nc.tensor.matmul(pr[:, :], b2c[:, k:k + 1],
                                     w1_raw[:, (k * DM + cc * D):(k * DM + (cc + 1) * D)],
                                     start=(k == 0), stop=(k == KD - 1))
                nc.scalar.copy(rowm[:, cc * D:(cc + 1) * D], pr[:, :])
            if use_m1:
                m1row = sb.tile([1, DM], f32, name="m1row", tag="m1row")
                nc.sync.dma_start(out=m1row, in_=mlp_b1[:].rearrange("(o f) -> o f", o=1))
                nc.vector.tensor_add(rowm, rowm, m1row)
            m1cols = persist.tile([128, 12], f32, tag="m1cols")
            for t in range(12):
                ptc = ps_pre.tile([128, 1], f32, name="ptm", tag="pt")
                nc.tensor.transpose(ptc[:, :], rowm[0:1, t * 128:(t + 1) * 128],
                                    identf[0:1, 0:1])
                nc.vector.tensor_copy(m1cols[:, t:t + 1], ptc[:, :])

            qkT = [persist.tile([128, L], bf16, name=f"qkT{t}", tag=f"qkT{t}")
                   for t in range(6)]
            vsb = [persist.tile([128, 6 * (HD + 1)], bf16, name=f"v{j}", tag=f"v{j}")
                   for j in range(IT)]
            attnT = [persist.tile([128, L], bf16, name=f"attnT{k}", tag=f"attnT{k}")
                     for k in range(KD)]

            def emit_qkT(t, pool):
                pq = pool.tile([128, L], f32, name="pq", tag="pq")
                for icc in range(IC):
                    for k in range(KD):
                        nc.tensor.matmul(pq[:, icc * 512:(icc + 1) * 512],
                                         qkvw_sc[:, (k * DQ + t * 128):(k * DQ + (t + 1) * 128)],
                                         lnT[k][:, icc * 512:(icc + 1) * 512],
                                         start=(k == 0), stop=(k == KD - 1))
                if t % 2 == 0:
                    nc.vector.tensor_scalar(out=qkT[t][:, :], in0=pq[:, :],
                                            scalar1=qkcols[:, t:t + 1],
                                            scalar2=None, op0=ALU.add)
                else:
                    nc.scalar.activation(out=qkT[t][:, :], in_=pq[:, :],
                                         func=ACTF.Identity,
                                         bias=qkcols[:, t:t + 1], scale=1.0)

            def emit_v(i, pool):
                pv = pool.tile([128, D], f32, name="pv", tag="pv")
                for k in range(KD):
                    nc.tensor.matmul(pv[:, :],
                                     lnT[k][:, i * 128:(i + 1) * 128],
                                     qkvw_sc[:, k * DQ + 768:k * DQ + 1152],
                                     start=(k == 0), stop=(k == KD - 1))
                vview = vsb[i][:, :].rearrange("p (h c) -> p h c", c=HD + 1)
                pvview = pv[:, :].rearrange("p (h c) -> p h c", c=HD)
                nc.vector.tensor_add(vview[:, :, 0:HD], pvview,
                                     vb_bc[:, :].rearrange("p (h c) -> p h c", c=HD))
                nc.gpsimd.tensor_copy(vview[:, :, HD:HD + 1], ones6_f)

            # qkT pair 0 + all v before attention (dense PE block warms HAM)
            emit_qkT(0, ps_pre)
            emit_qkT(3, ps_pre)
            for i in range(IT):
                emit_v(i, ps_pre)
            ps_pre_cm.__exit__(None, None, None)

            # ---------------- attention ----------------
            # pss shared tile: head a -> cols 0:512 (this ic), head b ->
            # cols 512:1024, packed via tile_position rows. po per head per
            # ic half. qkT for pairs 1, 2 emitted inside pairs 0, 1.
            with tc.tile_pool(name="ps_s", bufs=1, space="PSUM") as ps_s, \
                 tc.tile_pool(name="ps_o", bufs=2, space="PSUM") as ps_o, \
                 tc.tile_pool(name="ps_x2", bufs=1, space="PSUM") as ps_x2, \
                 tc.tile_pool(name="espool", bufs=3) as espool:
                deferred = {0: [lambda: emit_qkT(1, ps_x2), lambda: emit_qkT(4, ps_x2)],
                            1: [lambda: emit_qkT(2, ps_x2), lambda: emit_qkT(5, ps_x2)]}
                for tq in range(3):
                    ha, hb = 2 * tq, 2 * tq + 1
                    defs = deferred.get(tq, [])
                    for icc in range(IC):
                        po_a = ps_o.tile([HD + 1, 512], f32, name="po_a", tag="po_a")
                        po_b = ps_o.tile([HD + 1, 512], f32, name="po_b", tag="po_b")
                        for jt in range(IT):
                            pss = ps_s.tile([128, L], f32, name="pss", tag="pss")
                            nc.tensor.matmul(pss[:, 0:512],
                                             qkT[3 + tq][0:64, jt * 128:(jt + 1) * 128],
                                             qkT[tq][0:64, icc * 512:(icc + 1) * 512],
                                             start=True, stop=True, tile_position=(0, 0))
                            nc.tensor.matmul(pss[:, 512:1024],
                                             qkT[3 + tq][64:128, jt * 128:(jt + 1) * 128],
                                             qkT[tq][64:128, icc * 512:(icc + 1) * 512],
                                             start=True, stop=True, tile_position=(64, 0))
                            es_a = espool.tile([128, 512], i16, name="es_a", tag="es_a")
                            es_b = espool.tile([128, 512], i16, name="es_b", tag="es_b")
                            if jt % 2 == 0:
                                nc.scalar.activation(out=es_a[:, :].bitcast(bf16),
                                                     in_=pss[:, 0:512], func=ACTF.Exp,
                                                     scale=SCALE)
                                nc.vector.tensor_scalar(out=es_b, in0=pss[:, 512:1024],
                                                        scalar1=A_SCH, scalar2=B_SCH,
                                                        op0=ALU.mult, op1=ALU.add)
                            else:
                                nc.vector.tensor_scalar(out=es_a, in0=pss[:, 0:512],
                                                        scalar1=A_SCH, scalar2=B_SCH,
                                                        op0=ALU.mult, op1=ALU.add)
                                nc.scalar.activation(out=es_b[:, :].bitcast(bf16),
                                                     in_=pss[:, 512:1024], func=ACTF.Exp,
                                                     scale=SCALE)
                            nc.tensor.matmul(po_a[:, :],
                                             vsb[jt][:, ha * (HD + 1):(ha + 1) * (HD + 1)],
                                             es_a[:, :].bitcast(bf16),
                                             start=(jt == 0), stop=(jt == IT - 1))
                            nc.tensor.matmul(po_b[:, :],
                                             vsb[jt][:, hb * (HD + 1):(hb + 1) * (HD + 1)],
                                             es_b[:, :].bitcast(bf16),
                                             start=(jt == 0), stop=(jt == IT - 1))
                            if jt == 3 and defs:
                                defs.pop(0)()
                        # normalize -> attnT rows (a: 0:64, b: 64:128)
                        dn_a = sb.tile([1, 512], f32, name="dn_a", tag="dn_a", bufs=2)
                        nc.vector.tensor_copy(dn_a, po_a[HD:HD + 1, :])
                        dn_b = sb.tile([1, 512], f32, name="dn_b", tag="dn_b", bufs=2)
                        nc.vector.tensor_copy(dn_b, po_b[HD:HD + 1, :])
                        rcp_a = sb.tile([1, 512], f32, name="rcp_a", tag="rcp_a", bufs=2)
                        nc.vector.reciprocal_approx_fast(rcp_a, dn_a)
                        rcp_b = sb.tile([1, 512], f32, name="rcp_b", tag="rcp_b", bufs=2)
                        nc.vector.reciprocal_approx_fast(rcp_b, dn_b)
                        rcpb_a = sb.tile([HD, 512], f32, name="rcpb_a", tag="rcpb_a", bufs=2)
                        nc.gpsimd.partition_broadcast(rcpb_a, rcp_a[:1, :])
                        rcpb_b = sb.tile([HD, 512], f32, name="rcpb_b", tag="rcpb_b", bufs=2)
                        nc.gpsimd.partition_broadcast(rcpb_b, rcp_b[:1, :])
                        nc.vector.tensor_mul(
                            attnT[tq][0:HD, icc * 512:(icc + 1) * 512],
                            po_a[0:HD, :], rcpb_a)
                        nc.vector.tensor_mul(
                            attnT[tq][HD:128, icc * 512:(icc + 1) * 512],
                            po_b[0:HD, :], rcpb_b)
                    while defs:
                        defs.pop(0)()

            early_cm.__exit__(None, None, None)

            # -------- proj (g1-folded) + residual 1 + LN2 + transposes ------
            x1t = [persist.tile([128, D], f32, name=f"x1_{i}", tag=f"x1_{i}")
                   for i in range(IT)]
            h2T = [persist.tile([128, L], bf16, name=f"h2T{k}", tag=f"h2T{k}")
                   for k in range(KD)]
            if use_pb:
                pb_bc = persist.tile([128, D], f32, tag="pb_bc")
                pbrow = sb.tile([1, D], f32, name="pbrow", tag="pbrow")
                nc.sync.dma_start(out=pbrow, in_=proj_b[:].rearrange("(o f) -> o f", o=1))
                nc.gpsimd.partition_broadcast(pb_bc, pbrow[:1, :])
                nc.gpsimd.tensor_mul(pb_bc, pb_bc, g1bc)
            with tc.tile_pool(name="ps_p", bufs=4, space="PSUM") as ps_p:
                for i in range(IT):
                    py = ps_p.tile([128, D], f32, name="py", tag="py")
                    for k in range(KD):
                        nc.tensor.matmul(py[:, :],
                                         attnT[k][:, i * 128:(i + 1) * 128],
                                         projw_sc[:, k * D:(k + 1) * D],
                                         start=(k == 0), stop=(k == KD - 1))
                    pys = hpool.tile([128, D], f32, name="pys", tag="pys")
                    if use_pb:
                        nc.vector.tensor_add(pys, py[:, :], pb_bc)
                    else:
                        copy_engs[i % 2](pys, py[:, :])
                    nc.gpsimd.tensor_add(x1t[i], xt[i], pys)
                    ln2 = hpool.tile([128, D], bf16, name="ln2", tag="h2")
                    _layernorm(nc, sb, x1t[i], eps_t, ln2)
                    for k in range(KD):
                        pt = ps_p.tile([128, 128], bf16, name="pt2", tag="pt2")
                        nc.tensor.transpose(pt[:, :], ln2[:, k * 128:(k + 1) * 128],
                                            identb[:, :])
                        copy_engs[(i + k) % 2](h2T[k][:, i * 128:(i + 1) * 128], pt[:, :])

            # ---------------- MLP ----------------
            with tc.tile_pool(name="mlp1", bufs=1) as mp1, \
                 tc.tile_pool(name="ps_m", bufs=2, space="PSUM") as ps_m, \
                 tc.tile_pool(name="ps_m2", bufs=2, space="PSUM") as ps_m2:
                siluT = [mp1.tile([128, L], bf16, name=f"siluT{t}", tag=f"siluT{t}")
                         for t in range(12)]
                if use_m2:
                    m2_bc = persist.tile([128, D], f32, tag="m2_bc")
                    m2row = sb.tile([1, D], f32, name="m2row", tag="m2row")
                    nc.sync.dma_start(out=m2row, in_=mlp_b2[:].rearrange("(o f) -> o f", o=1))
                    nc.gpsimd.partition_broadcast(m2_bc, m2row[:1, :])
                    nc.gpsimd.tensor_mul(m2_bc, m2_bc, g2bc)

                for t in range(12):
                    pa = ps_m.tile([128, L], f32, name="pa", tag="m")
                    for icc in range(IC):
                        for k in range(KD):
                            nc.tensor.matmul(pa[:, icc * 512:(icc + 1) * 512],
                                             w1_sc[:, (k * DM + t * 128):(k * DM + (t + 1) * 128)],
                                             h2T[k][:, icc * 512:(icc + 1) * 512],
                                             start=(k == 0), stop=(k == KD - 1))
                    nc.scalar.activation(out=siluT[t][:, :], in_=pa[:, :],
                                         func=ACTF.Silu,
                                         bias=m1cols[:, t:t + 1], scale=1.0)

                # mlp2 (g2-folded) + residual + store
                for i in range(IT):
                    py2 = ps_m2.tile([128, D], f32, name="py2", tag="m2")
                    for k in range(12):
                        nc.tensor.matmul(py2[:, :],
                                         siluT[k][:, i * 128:(i + 1) * 128],
                                         w2_sc[:, k * D:(k + 1) * D],
                                         start=(k == 0), stop=(k == 11))
                    t2 = hpool.tile([128, D], f32, name="t2", tag="t2")
                    if use_m2:
                        nc.vector.tensor_add(t2, py2[:, :], m2_bc)
                    else:
                        copy_engs[i % 2](t2, py2[:, :])
                    ot = hpool.tile([128, D], f32, name="ot", tag="ot")
                    nc.gpsimd.tensor_add(ot, x1t[i], t2)
                    nc.sync.dma_start(out=out[i * 128:(i + 1) * 128, :], in_=ot)

    nc.compile()
    return nc


def _get_nc(flags):
    if flags not in _cache:
        _cache[flags] = build(flags)
    return _cache[flags]


def _prep(a, dt=np.float32):
    return np.ascontiguousarray(np.asarray(a, np.float32).astype(dt))


def kernel(x, cond, qkv_w, qkv_b, proj_w, proj_b, mlp_w1, mlp_b1, mlp_w2, mlp_b2,
           cond_w, cond_b, num_heads):
    x = np.asarray(x, np.float32)
    cond = np.asarray(cond, np.float32)
    qkv_b = np.asarray(qkv_b, np.float32)
    proj_b = np.asarray(proj_b, np.float32)
    mlp_b1 = np.asarray(mlp_b1, np.float32)
    mlp_b2 = np.asarray(mlp_b2, np.float32)
    cond_b = np.asarray(cond_b, np.float32)
    assert int(num_heads) == H and x.shape == (B, L, D)

    flags = (bool(cond_b.any()), bool(qkv_b.any()), bool(proj_b.any()),
             bool(mlp_b1.any()), bool(mlp_b2.any()))
    nc = _get_nc(flags)

    bf = ml_dtypes.bfloat16
    shared = {
        "cond_wT": _prep(np.asarray(cond_w, np.float32).T, bf),
        "qkv_wT": _prep(np.asarray(qkv_w, np.float32).T, bf),
        "proj_wT": _prep(np.asarray(proj_w, np.float32).T, bf),
        "w1T": _prep(np.asarray(mlp_w1, np.float32).T, bf),
        "w2T": _prep(np.asarray(mlp_w2, np.float32).T, bf),
    }
    if flags[0]:
        shared["cond_b"] = cond_b
    if flags[1]:
        shared["qkv_b"] = qkv_b
    if flags[2]:
        shared["proj_b"] = proj_b
    if flags[3]:
        shared["mlp_b1"] = mlp_b1
    if flags[4]:
        shared["mlp_b2"] = mlp_b2

    in_maps = [dict(shared, xb=np.ascontiguousarray(x[b]), cond=np.ascontiguousarray(cond[b]))
               for b in range(B)]
    res = run_bass_kernel_spmd(nc, in_maps, list(range(B)))
    return np.stack([res.results[b]["out"] for b in range(B)], axis=0)
